# revision 16
# baseline (speedup 1.0000x reference)
"""Trainium2 Bass kernel for BackboneR3Denoiser (gnn_message_passing).

Sharding: data-parallel over proteins; 2 cores per protein, each core owns
512 of the protein's 1024 nodes for the edge/update work and replicates the
cheap per-node table build.

Device kernel (per layer, SPMD over 8 cores):
  - l0 time-embed matmul, SO3 node features, per-node value/q/s table
  - indirect-DMA gather of neighbor records, attention softmax, weighted
    aggregation, output SO3 linears, FFN, gated coordinate/backbone updates.
Host (exact jax-CPU reproduction of the reference's RNG-dependent sampling):
  - KNN + inverse-cubic Gumbel edge sampling per layer (argsort + threefry)
  - per-edge rbf/posemb MLP bias term (ebias) and validity mask.
"""

import numpy as np

B, L, KNN, INV = 4, 1024, 30, 10
N = B * L
K = KNN + INV          # 40
CB, NB, NL = 32, 3, 4
SPH = CB + NB          # 35
H = 8                  # attention heads
REC = 304              # table record: [q 0:8 | v 8:296 | s 296:304]
M = 512                # nodes owned per core
LMAP = [0, 1, 1, 1, 2, 2, 2, 2, 2]

_KHALF = 10            # gather k in groups per node tile
_NG = K // _KHALF      # 4 groups

_CACHE = {}


def _build_kernel():
    import concourse.bacc as bacc
    import concourse.bass as bass
    import concourse.mybir as mybir
    from concourse.tile import TileContext
    from concourse.masks import make_identity

    f32 = mybir.dt.float32
    i32 = mybir.dt.int32
    AX = mybir.AxisListType
    OP = mybir.AluOpType
    AF = mybir.ActivationFunctionType

    nc = bacc.Bacc("TRN2", target_bir_lowering=False, debug=False)

    # ------------- I/O -------------
    featsT = nc.dram_tensor("featsT", [9, CB, L], f32, kind="ExternalInput")
    nmask_full = nc.dram_tensor("nmask_full", [1, L], f32, kind="ExternalInput")
    nmask_own = nc.dram_tensor("nmask_own", [3, M], f32, kind="ExternalInput")
    bbT_full = nc.dram_tensor("bbT_full", [3, 3, L], f32, kind="ExternalInput")
    bb_own = nc.dram_tensor("bb_own", [3, 3, M], f32, kind="ExternalInput")
    XT_own = nc.dram_tensor("XT_own", [3, M], f32, kind="ExternalInput")
    tvec = nc.dram_tensor("tvec", [CB, 1], f32, kind="ExternalInput")
    nb_in = nc.dram_tensor("nb", [M, K], i32, kind="ExternalInput")
    self_idx = nc.dram_tensor("self_idx", [M, 1], i32, kind="ExternalInput")
    ebias = nc.dram_tensor("ebias", [M, K * H], f32, kind="ExternalInput")
    eWf = nc.dram_tensor("eWf", [CB, CB], f32, kind="ExternalInput")
    Wv_l = [nc.dram_tensor(f"Wv{l}", [SPH, CB], f32, kind="ExternalInput") for l in range(3)]
    Wq = nc.dram_tensor("Wq", [SPH, H], f32, kind="ExternalInput")
    Ws = nc.dram_tensor("Ws", [SPH, H], f32, kind="ExternalInput")
    Wo_l = [nc.dram_tensor(f"Wo{l}", [CB, CB], f32, kind="ExternalInput") for l in range(3)]
    Wf1 = nc.dram_tensor("Wf1", [CB, CB], f32, kind="ExternalInput")
    Wf2 = nc.dram_tensor("Wf2", [CB, CB], f32, kind="ExternalInput")
    Wx1 = nc.dram_tensor("Wx1", [CB, 1], f32, kind="ExternalInput")
    Wg = nc.dram_tensor("Wg", [CB, 1], f32, kind="ExternalInput")
    Wb1 = nc.dram_tensor("Wb1", [CB, 3], f32, kind="ExternalInput")
    bv_b = nc.dram_tensor("bv_b", [128, CB], f32, kind="ExternalInput")
    bo0 = nc.dram_tensor("bo0", [CB, 1], f32, kind="ExternalInput")
    bf1 = nc.dram_tensor("bf1", [CB, 1], f32, kind="ExternalInput")
    bf2 = nc.dram_tensor("bf2", [CB, 1], f32, kind="ExternalInput")
    bg1 = nc.dram_tensor("bg1", [1, 1], f32, kind="ExternalInput")

    featsT_out = nc.dram_tensor("featsT_out", [9, CB, M], f32, kind="ExternalOutput")
    XT_out = nc.dram_tensor("XT_out", [3, M], f32, kind="ExternalOutput")
    bbT_out = nc.dram_tensor("bbT_out", [3, 3, M], f32, kind="ExternalOutput")

    table_d = nc.dram_tensor("table_d", [L, REC], f32)

    with TileContext(nc) as tc:
        with (
            tc.tile_pool(name="const", bufs=1) as cp,
            tc.tile_pool(name="sb", bufs=2) as sb,
            tc.tile_pool(name="gath", bufs=5) as gp,
            tc.tile_pool(name="sb1", bufs=1) as sb1,
            tc.tile_pool(name="psA", bufs=2, space="PSUM") as psA,   # [128,320]
            tc.tile_pool(name="psB", bufs=2, space="PSUM") as psB,   # [32,512]
            tc.tile_pool(name="psC", bufs=2, space="PSUM") as psC,   # misc small
        ):
            ident = cp.tile([128, 128], f32)
            make_identity(nc, ident[:])

            def load_const(drt, shape):
                t = cp.tile(shape, drt.ap().dtype, tag=f"c_{drt.name}")
                nc.sync.dma_start(out=t[:], in_=drt[:].rearrange(
                    "a b c -> (a b) c") if len(drt.shape) == 3 else drt[:])
                return t

            w_eWf = load_const(eWf, [CB, CB])
            w_Wv = [load_const(Wv_l[l], [SPH, CB]) for l in range(3)]
            w_Wq = load_const(Wq, [SPH, H])
            w_Ws = load_const(Ws, [SPH, H])
            w_Wo = [load_const(Wo_l[l], [CB, CB]) for l in range(3)]
            w_Wf1 = load_const(Wf1, [CB, CB])
            w_Wf2 = load_const(Wf2, [CB, CB])
            w_Wx1 = load_const(Wx1, [CB, 1])
            w_Wg = load_const(Wg, [CB, 1])
            w_Wb1 = load_const(Wb1, [CB, 3])
            w_bvb = load_const(bv_b, [128, CB])
            w_bo0 = load_const(bo0, [CB, 1])
            w_bf1 = load_const(bf1, [CB, 1])
            w_bf2 = load_const(bf2, [CB, 1])
            w_bg = load_const(bg1, [1, 1])
            w_tvec = load_const(tvec, [CB, 1])
            nm_full = load_const(nmask_full, [1, L])
            nm_own = load_const(nmask_own, [3, M])
            bo_a = []
            xo_a = []
            for a in range(3):
                t1 = cp.tile([3, M], f32, tag=f"bo{a}")
                nc.sync.dma_start(out=t1[:], in_=bb_own[a])
                bo_a.append(t1)
                t2 = cp.tile([1, M], f32, tag=f"xo{a}")
                nc.sync.dma_start(out=t2[:], in_=XT_own[a:a + 1, :])
                xo_a.append(t2)

            # ---------------- stage 1: node tables ----------------
            nfT = cp.tile([SPH, 9, L], f32)
            for m in range(9):
                nc.sync.dma_start(out=nfT[:CB, m, :], in_=featsT[m])
            nc.vector.memset(nfT[CB:SPH, :, :], 0.0)
            for a in range(3):
                nc.sync.dma_start(out=nfT[CB:SPH, 1 + a, :], in_=bbT_full[a])
            nc.sync.dma_start(out=nfT[SPH - 1:SPH, 0, :], in_=nmask_full[:])

            # inv = feats0 @ eW[:32] + tvec  (overwrites nfT[:, 0])
            for c in range(2):
                sl = slice(c * 512, (c + 1) * 512)
                pi = psB.tile([CB, 512], f32, tag="b512")
                nc.tensor.matmul(pi[:], lhsT=w_eWf[:], rhs=nfT[:CB, 0, sl],
                                 start=True, stop=True)
                nc.vector.tensor_add(out=nfT[:CB, 0, sl], in0=pi[:],
                                     in1=w_tvec[:].to_broadcast([CB, 512]))

            # node record table: q | v | s
            for c8 in range(8):
                ns = slice(c8 * 128, (c8 + 1) * 128)
                pt = psA.tile([128, REC], f32, tag="tab")
                nc.tensor.matmul(pt[:, 0:H], lhsT=nfT[:, 0, ns], rhs=w_Wq[:],
                                 start=True, stop=True)
                for m in range(9):
                    c0 = H + m * CB
                    nc.tensor.matmul(pt[:, c0:c0 + CB], lhsT=nfT[:, m, ns],
                                     rhs=w_Wv[LMAP[m]][:], start=True, stop=True)
                nc.tensor.matmul(pt[:, 296:304], lhsT=nfT[:, 0, ns], rhs=w_Ws[:],
                                 start=True, stop=True)
                tabt = sb.tile([128, REC], f32, tag="tabt")
                nc.vector.tensor_copy(out=tabt[:, 0:304], in_=pt[:, 0:304])
                nc.vector.tensor_add(out=tabt[:, H:H + CB], in0=tabt[:, H:H + CB],
                                     in1=w_bvb[:])
                nc.sync.dma_start(out=table_d[ns, 0:304], in_=tabt[:, 0:304])

            # ---------------- stage 2: edges ----------------
            aggT = cp.tile([CB, 9, M], f32)
            for t in range(4):
                rs = slice(t * 128, (t + 1) * 128)
                nbt = sb.tile([128, K], i32, tag="nbt")
                nc.sync.dma_start(out=nbt[:], in_=nb_in[rs, :])
                sft = sb.tile([128, 1], i32, tag="sft")
                nc.sync.dma_start(out=sft[:], in_=self_idx[rs, :])
                ebt = sb1.tile([128, K * H], f32, tag="ebt")
                nc.sync.dma_start(out=ebt[:], in_=ebias[rs, :])

                gself = sb.tile([128, REC], f32, tag="gself")
                nc.gpsimd.indirect_dma_start(
                    out=gself[:], out_offset=None, in_=table_d[:],
                    in_offset=bass.IndirectOffsetOnAxis(ap=sft[:, :1], axis=0))

                ghs = []
                for hf in range(_NG):
                    gh = gp.tile([128, _KHALF, REC], f32, tag="g")
                    for kk in range(_KHALF):
                        k = hf * _KHALF + kk
                        nc.gpsimd.indirect_dma_start(
                            out=gh[:, kk, :], out_offset=None, in_=table_d[:],
                            in_offset=bass.IndirectOffsetOnAxis(ap=nbt[:, k:k + 1], axis=0))
                    ghs.append(gh)

                # logits [128, K, H] = q_gathered + s_self + ebias
                Lt = sb1.tile([128, K, H], f32, tag="Lt")
                sview = gself[:, 296:304].unsqueeze(1).broadcast_to([128, _KHALF, H])
                for hf in range(_NG):
                    nc.vector.tensor_add(out=Lt[:, hf * _KHALF:(hf + 1) * _KHALF, :],
                                         in0=ghs[hf][:, :, 0:H], in1=sview)
                nc.vector.tensor_add(out=Lt[:], in0=Lt[:],
                                     in1=ebt[:].rearrange("p (k h) -> p k h", k=K))

                mx = sb1.tile([128, H], f32, tag="mx")
                nc.vector.tensor_reduce(out=mx[:],
                                        in_=Lt[:].rearrange("p k h -> p h k"),
                                        axis=AX.X, op=OP.max)
                ex = sb1.tile([128, K, H], f32, tag="ex")
                nc.vector.tensor_tensor(out=ex[:], in0=Lt[:],
                                        in1=mx[:].unsqueeze(1).broadcast_to([128, K, H]),
                                        op=OP.subtract)
                nc.scalar.activation(out=ex[:], in_=ex[:], func=AF.Exp)
                sm = sb1.tile([128, H], f32, tag="sm")
                nc.vector.tensor_reduce(out=sm[:],
                                        in_=ex[:].rearrange("p k h -> p h k"),
                                        axis=AX.X, op=OP.add)
                nc.vector.tensor_scalar(sm[:], sm[:], 1e-9, scalar2=None, op0=OP.add)
                rc = sb1.tile([128, H], f32, tag="rc")
                nc.vector.reciprocal(out=rc[:], in_=sm[:])
                al = sb1.tile([128, K, H], f32, tag="al")
                nc.vector.tensor_tensor(out=al[:], in0=ex[:],
                                        in1=rc[:].unsqueeze(1).broadcast_to([128, K, H]),
                                        op=OP.mult)
                al32 = sb1.tile([128, K, CB], f32, tag="al32")
                nc.vector.tensor_copy(
                    out=al32[:].rearrange("p k (h c) -> p k h c", h=H),
                    in_=al[:].unsqueeze(3).broadcast_to([128, K, H, 4]))

                # weighted aggregation over k
                agg = sb1.tile([128, 288], f32, tag="agg")
                ahalf = sb1.tile([128, 288], f32, tag="ahalf")
                for hf in range(_NG):
                    alv = al32[:, hf * _KHALF:(hf + 1) * _KHALF, :]
                    gv = ghs[hf][:, :, H:296].rearrange("p k (m w) -> p k m w", m=9)
                    nc.vector.tensor_tensor(
                        out=gv, in0=gv,
                        in1=alv.unsqueeze(2).broadcast_to([128, _KHALF, 9, CB]),
                        op=OP.mult)
                    dst = agg if hf == 0 else ahalf
                    nc.vector.tensor_reduce(
                        out=dst[:],
                        in_=ghs[hf][:, :, H:296].rearrange("p k j -> p j k"),
                        axis=AX.X, op=OP.add)
                    if hf > 0:
                        nc.vector.tensor_add(out=agg[:], in0=agg[:], in1=ahalf[:])

                # transpose agg -> aggT[:, m, own-slice]
                for m in range(9):
                    ptr = psC.tile([CB, 128], f32, tag="small")
                    nc.tensor.transpose(out=ptr[:], in_=agg[:, m * CB:(m + 1) * CB],
                                        identity=ident[:])
                    nc.vector.tensor_copy(out=aggT[:, m, rs], in_=ptr[:])

            # ---------------- stage 3: outputs (feature-major) ----------------
            outT = cp.tile([CB, 9, M], f32)
            for m in range(9):
                po = psB.tile([CB, M], f32, tag="b512")
                nc.tensor.matmul(po[:], lhsT=w_Wo[LMAP[m]][:], rhs=aggT[:, m, :],
                                 start=True, stop=True)
                if m == 0:
                    nc.vector.tensor_add(out=outT[:, 0, :], in0=po[:],
                                         in1=w_bo0[:].to_broadcast([CB, M]))
                else:
                    nc.vector.tensor_copy(out=outT[:, m, :], in_=po[:])

            # FFN on m=0
            ph = psB.tile([CB, M], f32, tag="b512")
            nc.tensor.matmul(ph[:], lhsT=w_Wf1[:], rhs=outT[:, 0, :], start=True, stop=True)
            h1 = sb1.tile([CB, M], f32, tag="h1")
            nc.scalar.activation(out=h1[:], in_=ph[:], func=AF.Relu, bias=w_bf1[:, :1])
            pf = psB.tile([CB, M], f32, tag="b512")
            nc.tensor.matmul(pf[:], lhsT=w_Wf2[:], rhs=h1[:], start=True, stop=True)
            f2 = sb1.tile([CB, M], f32, tag="f2")
            nc.vector.tensor_add(out=f2[:], in0=pf[:], in1=w_bf2[:].to_broadcast([CB, M]))
            nc.vector.tensor_add(out=outT[:, 0, :], in0=outT[:, 0, :], in1=f2[:])

            # gate & coordinate update
            upd_a = []
            for a in range(3):
                pu = psC.tile([1, M], f32, tag="small")
                nc.tensor.matmul(pu[:], lhsT=w_Wx1[:], rhs=outT[:, 1 + a, :],
                                 start=True, stop=True)
                ut = sb1.tile([1, M], f32, tag=f"upd{a}")
                nc.vector.tensor_copy(out=ut[:], in_=pu[:])
                upd_a.append(ut)
            pg = psC.tile([1, M], f32, tag="small")
            nc.tensor.matmul(pg[:], lhsT=w_Wg[:], rhs=outT[:, 0, :], start=True, stop=True)
            gT = sb1.tile([1, M], f32, tag="gT")
            nc.scalar.activation(out=gT[:], in_=pg[:], func=AF.Exp, bias=w_bg[:1, :1])
            nc.vector.tensor_scalar(gT[:], gT[:], 1.0, scalar2=None, op0=OP.add)
            nc.scalar.activation(out=gT[:], in_=gT[:], func=AF.Ln)

            for a in range(3):
                xu = sb1.tile([1, M], f32, tag=f"xu{a}")
                nc.vector.tensor_tensor(out=xu[:], in0=upd_a[a][:], in1=gT[:1, :],
                                        op=OP.mult)
                nc.vector.tensor_tensor(out=xu[:], in0=xu[:], in1=nm_own[:1, :],
                                        op=OP.mult)
                nc.vector.tensor_add(out=xu[:], in0=xu[:], in1=xo_a[a][:])
                nc.sync.dma_start(out=XT_out[a:a + 1, :], in_=xu[:])

            # backbone update
            for a in range(3):
                pb = psC.tile([3, M], f32, tag="small")
                nc.tensor.matmul(pb[:], lhsT=w_Wb1[:], rhs=outT[:, 1 + a, :],
                                 start=True, stop=True)
                ub = sb1.tile([3, M], f32, tag="ub")
                nc.vector.tensor_tensor(out=ub[:], in0=pb[:], in1=nm_own[:],
                                        op=OP.mult)
                nc.vector.tensor_add(out=ub[:], in0=ub[:], in1=bo_a[a][:])
                nc.sync.dma_start(out=bbT_out[a], in_=ub[:])

            nc.sync.dma_start(out=featsT_out[:].rearrange("m d n -> d m n"), in_=outT[:])

    nc.compile()
    return nc


def _get_nc():
    if "nc" not in _CACHE:
        _CACHE["nc"] = _build_kernel()
    return _CACHE["nc"]


# ----------------------------------------------------------------------------
# host-side exact reference pieces (jax CPU)
# ----------------------------------------------------------------------------

def _host_mod():
    if "host" in _CACHE:
        return _CACHE["host"]
    import jax
    import jax.numpy as jnp
    cpu = jax.devices("cpu")[0]
    _CACHE["host"] = (jax, jnp, cpu)
    return _CACHE["host"]


def _sample_edges_host(X, x_mask, layer_i):
    """Exact replica of reference.sample_edges, local indices [B, L, K]."""
    jax, jnp, cpu = _host_mod()
    with jax.default_device(cpu):
        key = jax.random.fold_in(jax.random.key(42), layer_i)
        Xb = jnp.where(x_mask[:, None], 1e9, X).reshape(B, L, 3)

        def per(Xp, k):
            d = jnp.linalg.norm(Xp[:, None] - Xp[None], axis=-1)
            idx = jnp.argsort(d, axis=-1)
            sd = jnp.take_along_axis(d, idx, -1)
            knn = idx[:, :KNN]
            u = jax.random.uniform(k, (L, L - KNN), minval=1e-6, maxval=1.0 - 1e-6)
            logp = -3.0 * jnp.log(jnp.maximum(sd[:, KNN:], 1e-9)) - jnp.log(-jnp.log(u))
            _, top = jax.lax.top_k(logp, INV)
            samp = jnp.take_along_axis(idx[:, KNN:], top, -1)
            return jnp.concatenate([knn, samp], -1)

        nb = jax.vmap(per)(Xb, jax.random.split(key, B))
        return np.asarray(nb).astype(np.int32)       # [B, L, K] local


def _edge_bias_host(X, nb_local, We_i, be_i, Wa3_i, ba_i):
    """ebias[n,k,h] = relu([rbf|posemb] @ We + be) @ Wa[70:] + ba, with -1e9
    folded in for invalid edges. X: [N,3] centered; nb_local: [B,L,K]."""
    jax, jnp, cpu = _host_mod()
    with jax.default_device(cpu):
        nbg = (nb_local.astype(np.int64)
               + (np.arange(B)[:, None, None] * L)).reshape(-1)
        slf = np.repeat(np.arange(N), K)
        Xj = jnp.asarray(X)
        dvec = Xj[nbg] - Xj[slf]
        dist = jnp.linalg.norm(dvec, axis=-1)
        valid = (dist > 0.1) & (dist < 1e8)
        mu = jnp.linspace(0.0, 20.0, 16)
        sig = 20.0 / 16.0
        rbf = jnp.exp(-(((dist[:, None] - mu) / sig) ** 2))
        freq = jnp.exp(jnp.arange(0, 16, 2, dtype=jnp.float32)
                       * (-np.log(10000.0) / 16.0))
        diff = (nbg - slf).astype(np.int32)
        aa = jnp.asarray(diff)[:, None].astype(jnp.float32) * freq
        pe = jnp.concatenate([jnp.cos(aa), jnp.sin(aa)], -1)
        e = jax.nn.relu(jnp.concatenate([rbf, pe], -1) @ jnp.asarray(We_i)
                        + jnp.asarray(be_i))
        eb = e @ jnp.asarray(Wa3_i) + jnp.asarray(ba_i)
        eb = jnp.where(valid[:, None], eb, -1e9)
        return np.asarray(eb, dtype=np.float32).reshape(B, L, K * H)


def kernel(noised_bb, t, x_mask, noising_mask, kappa, tW1, tb1, tW2, tb2, eW, eb,
           We, be, Wa, ba, Wv, bv, Wo, bo, Wf1, bf1, Wf2, bf2, Wx, bx, Wg, bg,
           Wb, bbias):
    import os
    os.environ["BASS_NEVER_TRACE"] = "1"   # no NTFF hook on this axon client
    from concourse.bass_utils import run_bass_kernel_spmd

    jax, jnp, cpu = _host_mod()
    nc = _get_nc()

    noised_bb = np.asarray(noised_bb, dtype=np.float32)
    x_mask_np = np.asarray(x_mask)
    nmask_np = np.asarray(noising_mask)

    with jax.default_device(cpu):
        X0 = jnp.asarray(noised_bb[:, 1])
        w = (~jnp.asarray(x_mask_np)).astype(jnp.float32).reshape(B, L, 1)
        Xr = X0.reshape(B, L, 3)
        center = jnp.repeat((Xr * w).sum(1) / jnp.maximum(w.sum(1), 1.0), L, axis=0)
        X = np.asarray(X0 - center, dtype=np.float32)          # [N,3]
        tp = 2.0 * np.pi * jnp.asarray(t)[:, None] * jnp.asarray(kappa)
        ft = jnp.concatenate([jnp.cos(tp), jnp.sin(tp)], -1)
        et = jax.nn.relu(jax.nn.relu(ft @ jnp.asarray(tW1) + jnp.asarray(tb1))
                         @ jnp.asarray(tW2) + jnp.asarray(tb2))   # [B,64]
        tvec_np = np.asarray(et @ jnp.asarray(eW)[CB:] + jnp.asarray(eb),
                             dtype=np.float32)                  # [B,32]
    center_np = np.asarray(center, dtype=np.float32)

    bb_rel = noised_bb[:, [0, 2, 3]]                            # [N,3,3]
    # device states (per protein)
    featsT = [np.zeros((9, CB, L), np.float32) for _ in range(B)]
    bbT = [np.ascontiguousarray(bb_rel.reshape(B, L, 3, 3)[p].transpose(2, 1, 0))
           for p in range(B)]                                   # [a, j, n]
    XT = [np.ascontiguousarray(X.reshape(B, L, 3)[p].T) for p in range(B)]
    nmask_f = nmask_np.astype(np.float32).reshape(B, L)

    Wa_np = np.asarray(Wa, dtype=np.float32)
    core_ids = list(range(8))

    for i in range(NL):
        nb_local = _sample_edges_host(X, jnp.asarray(x_mask_np), i)  # [B,L,K]
        ebias_np = _edge_bias_host(X, nb_local,
                                   np.asarray(We)[i], np.asarray(be)[i],
                                   Wa_np[i][2 * SPH:], np.asarray(ba)[i])
        in_maps = []
        for c in core_ids:
            p, half = c // 2, c % 2
            sl = slice(half * M, (half + 1) * M)
            im = {
                "featsT": featsT[p],
                "nmask_full": nmask_f[p][None, :],
                "nmask_own": np.repeat(nmask_f[p][None, sl], 3, axis=0),
                "bbT_full": bbT[p],
                "bb_own": np.ascontiguousarray(bbT[p][:, :, sl]),
                "XT_own": np.ascontiguousarray(XT[p][:, sl]),
                "tvec": tvec_np[p][:, None],
                "nb": np.ascontiguousarray(nb_local[p, sl]),
                "self_idx": np.arange(half * M, (half + 1) * M,
                                      dtype=np.int32)[:, None],
                "ebias": np.ascontiguousarray(ebias_np[p, sl]),
                "eWf": np.asarray(eW, np.float32)[:CB],
                "Wq": Wa_np[i][:SPH],
                "Ws": Wa_np[i][SPH:2 * SPH],
                "Wf1": np.asarray(Wf1, np.float32)[i],
                "Wf2": np.asarray(Wf2, np.float32)[i],
                "Wx1": np.asarray(Wx, np.float32)[i][1],
                "Wg": np.asarray(Wg, np.float32)[i],
                "Wb1": np.asarray(Wb, np.float32)[i][1],
                "bv_b": np.repeat(np.asarray(bv, np.float32)[i][None, :], 128, 0),
                "bo0": np.asarray(bo, np.float32)[i][:, None],
                "bf1": np.asarray(bf1, np.float32)[i][:, None],
                "bf2": np.asarray(bf2, np.float32)[i][:, None],
                "bg1": np.asarray(bg, np.float32)[i].reshape(1, 1),
            }
            for l in range(3):
                im[f"Wv{l}"] = np.asarray(Wv, np.float32)[i][l]
                im[f"Wo{l}"] = np.asarray(Wo, np.float32)[i][l]
            in_maps.append(im)

        res = run_bass_kernel_spmd(nc, in_maps, core_ids=core_ids)
        _CACHE.setdefault("results", []).append(res)
        for c in core_ids:
            p, half = c // 2, c % 2
            sl = slice(half * M, (half + 1) * M)
            r = res.results[c]
            featsT[p][:, :, sl] = r["featsT_out"]
            XT[p][:, sl] = r["XT_out"]
            bbT[p][:, :, sl] = r["bbT_out"]
        X = np.concatenate([XT[p].T for p in range(B)], axis=0)

    den = np.zeros((N, 4, 3), np.float32)
    den[:, 1] = X + center_np
    bb_final = np.concatenate(
        [bbT[p].transpose(2, 1, 0) for p in range(B)], axis=0)  # [N, j, a]
    den[:, 0] = bb_final[:, 0]
    den[:, 2] = bb_final[:, 1]
    den[:, 3] = bb_final[:, 2]
    return den


# revision 17
# speedup vs baseline: 1.0595x; 1.0595x over previous
"""Trainium2 Bass kernel for BackboneR3Denoiser (gnn_message_passing).

Sharding: data-parallel over proteins; 2 cores per protein, each core owns
512 of the protein's 1024 nodes for the edge/update work and replicates the
cheap per-node table build.

Device kernel (per layer, SPMD over 8 cores):
  - l0 time-embed matmul, SO3 node features, per-node value/q/s table
  - indirect-DMA gather of neighbor records, attention softmax, weighted
    aggregation, output SO3 linears, FFN, gated coordinate/backbone updates.
Host (exact jax-CPU reproduction of the reference's RNG-dependent sampling):
  - KNN + inverse-cubic Gumbel edge sampling per layer (argsort + threefry)
  - per-edge rbf/posemb MLP bias term (ebias) and validity mask.
"""

import numpy as np

B, L, KNN, INV = 4, 1024, 30, 10
N = B * L
K = KNN + INV          # 40
CB, NB, NL = 32, 3, 4
SPH = CB + NB          # 35
H = 8                  # attention heads
REC = 304              # table record: [q 0:8 | v 8:296 | s 296:304]
M = 512                # nodes owned per core
LMAP = [0, 1, 1, 1, 2, 2, 2, 2, 2]

_KHALF = 10            # gather k in groups per node tile
_NG = K // _KHALF      # 4 groups

_CACHE = {}


def _build_kernel():
    import concourse.bacc as bacc
    import concourse.bass as bass
    import concourse.mybir as mybir
    from concourse.tile import TileContext
    from concourse.masks import make_identity

    f32 = mybir.dt.float32
    i32 = mybir.dt.int32
    AX = mybir.AxisListType
    OP = mybir.AluOpType
    AF = mybir.ActivationFunctionType

    nc = bacc.Bacc("TRN2", target_bir_lowering=False, debug=False)

    # ------------- I/O -------------
    featsT = nc.dram_tensor("featsT", [9, CB, L], f32, kind="ExternalInput")
    nmask_full = nc.dram_tensor("nmask_full", [1, L], f32, kind="ExternalInput")
    nmask_own = nc.dram_tensor("nmask_own", [3, M], f32, kind="ExternalInput")
    bbT_full = nc.dram_tensor("bbT_full", [3, 3, L], f32, kind="ExternalInput")
    bb_own = nc.dram_tensor("bb_own", [3, 3, M], f32, kind="ExternalInput")
    XT_own = nc.dram_tensor("XT_own", [3, M], f32, kind="ExternalInput")
    tvec = nc.dram_tensor("tvec", [CB, 1], f32, kind="ExternalInput")
    nb_in = nc.dram_tensor("nb", [M, K], i32, kind="ExternalInput")
    self_idx = nc.dram_tensor("self_idx", [M, 1], i32, kind="ExternalInput")
    ebias = nc.dram_tensor("ebias", [M, K * H], f32, kind="ExternalInput")
    eWf = nc.dram_tensor("eWf", [CB, CB], f32, kind="ExternalInput")
    Wv_l = [nc.dram_tensor(f"Wv{l}", [SPH, CB], f32, kind="ExternalInput") for l in range(3)]
    Wq = nc.dram_tensor("Wq", [SPH, H], f32, kind="ExternalInput")
    Ws = nc.dram_tensor("Ws", [SPH, H], f32, kind="ExternalInput")
    Wo_l = [nc.dram_tensor(f"Wo{l}", [CB, CB], f32, kind="ExternalInput") for l in range(3)]
    Wf1 = nc.dram_tensor("Wf1", [CB, CB], f32, kind="ExternalInput")
    Wf2 = nc.dram_tensor("Wf2", [CB, CB], f32, kind="ExternalInput")
    Wx1 = nc.dram_tensor("Wx1", [CB, 1], f32, kind="ExternalInput")
    Wg = nc.dram_tensor("Wg", [CB, 1], f32, kind="ExternalInput")
    Wb1 = nc.dram_tensor("Wb1", [CB, 3], f32, kind="ExternalInput")
    bv_b = nc.dram_tensor("bv_b", [128, CB], f32, kind="ExternalInput")
    bo0 = nc.dram_tensor("bo0", [CB, 1], f32, kind="ExternalInput")
    bf1 = nc.dram_tensor("bf1", [CB, 1], f32, kind="ExternalInput")
    bf2 = nc.dram_tensor("bf2", [CB, 1], f32, kind="ExternalInput")
    bg1 = nc.dram_tensor("bg1", [1, 1], f32, kind="ExternalInput")

    featsT_out = nc.dram_tensor("featsT_out", [9, CB, M], f32, kind="ExternalOutput")
    XT_out = nc.dram_tensor("XT_out", [3, M], f32, kind="ExternalOutput")
    bbT_out = nc.dram_tensor("bbT_out", [3, 3, M], f32, kind="ExternalOutput")

    table_d = nc.dram_tensor("table_d", [L, REC], f32)

    with TileContext(nc) as tc:
        with (
            tc.tile_pool(name="const", bufs=1) as cp,
            tc.tile_pool(name="sb", bufs=2) as sb,
            tc.tile_pool(name="gath", bufs=6) as gp,
            tc.tile_pool(name="sb1", bufs=1) as sb1,
            tc.tile_pool(name="psA", bufs=2, space="PSUM") as psA,   # [128,320]
            tc.tile_pool(name="psB", bufs=2, space="PSUM") as psB,   # [32,512]
            tc.tile_pool(name="psC", bufs=2, space="PSUM") as psC,   # misc small
        ):
            ident = cp.tile([128, 128], f32)
            make_identity(nc, ident[:])

            def load_const(drt, shape):
                t = cp.tile(shape, drt.ap().dtype, tag=f"c_{drt.name}")
                nc.sync.dma_start(out=t[:], in_=drt[:].rearrange(
                    "a b c -> (a b) c") if len(drt.shape) == 3 else drt[:])
                return t

            w_eWf = load_const(eWf, [CB, CB])
            w_Wv = [load_const(Wv_l[l], [SPH, CB]) for l in range(3)]
            w_Wq = load_const(Wq, [SPH, H])
            w_Ws = load_const(Ws, [SPH, H])
            w_Wo = [load_const(Wo_l[l], [CB, CB]) for l in range(3)]
            w_Wf1 = load_const(Wf1, [CB, CB])
            w_Wf2 = load_const(Wf2, [CB, CB])
            w_Wx1 = load_const(Wx1, [CB, 1])
            w_Wg = load_const(Wg, [CB, 1])
            w_Wb1 = load_const(Wb1, [CB, 3])
            w_bvb = load_const(bv_b, [128, CB])
            w_bo0 = load_const(bo0, [CB, 1])
            w_bf1 = load_const(bf1, [CB, 1])
            w_bf2 = load_const(bf2, [CB, 1])
            w_bg = load_const(bg1, [1, 1])
            w_tvec = load_const(tvec, [CB, 1])
            nm_full = load_const(nmask_full, [1, L])
            nm_own = load_const(nmask_own, [3, M])
            bo_a = []
            xo_a = []
            for a in range(3):
                t1 = cp.tile([3, M], f32, tag=f"bo{a}")
                nc.sync.dma_start(out=t1[:], in_=bb_own[a])
                bo_a.append(t1)
                t2 = cp.tile([1, M], f32, tag=f"xo{a}")
                nc.sync.dma_start(out=t2[:], in_=XT_own[a:a + 1, :])
                xo_a.append(t2)

            # ---------------- stage 1: node tables ----------------
            nfT = cp.tile([SPH, 9, L], f32, tag="bigA")
            for m in range(9):
                nc.sync.dma_start(out=nfT[:CB, m, :], in_=featsT[m])
            nc.vector.memset(nfT[CB:SPH, :, :], 0.0)
            for a in range(3):
                nc.sync.dma_start(out=nfT[CB:SPH, 1 + a, :], in_=bbT_full[a])
            nc.sync.dma_start(out=nfT[SPH - 1:SPH, 0, :], in_=nmask_full[:])

            # inv = feats0 @ eW[:32] + tvec  (overwrites nfT[:, 0])
            for c in range(2):
                sl = slice(c * 512, (c + 1) * 512)
                pi = psB.tile([CB, 512], f32, tag="b512")
                nc.tensor.matmul(pi[:], lhsT=w_eWf[:], rhs=nfT[:CB, 0, sl],
                                 start=True, stop=True)
                nc.vector.tensor_add(out=nfT[:CB, 0, sl], in0=pi[:],
                                     in1=w_tvec[:].to_broadcast([CB, 512]))

            # node record table: q | v | s
            for c8 in range(8):
                ns = slice(c8 * 128, (c8 + 1) * 128)
                pt = psA.tile([128, REC], f32, tag="tab")
                nc.tensor.matmul(pt[:, 0:H], lhsT=nfT[:, 0, ns], rhs=w_Wq[:],
                                 start=True, stop=True)
                for m in range(9):
                    c0 = H + m * CB
                    nc.tensor.matmul(pt[:, c0:c0 + CB], lhsT=nfT[:, m, ns],
                                     rhs=w_Wv[LMAP[m]][:], start=True, stop=True)
                nc.tensor.matmul(pt[:, 296:304], lhsT=nfT[:, 0, ns], rhs=w_Ws[:],
                                 start=True, stop=True)
                tabt = sb.tile([128, REC], f32, tag="tabt")
                nc.vector.tensor_copy(out=tabt[:, 0:304], in_=pt[:, 0:304])
                nc.vector.tensor_add(out=tabt[:, H:H + CB], in0=tabt[:, H:H + CB],
                                     in1=w_bvb[:])
                nc.sync.dma_start(out=table_d[ns, 0:304], in_=tabt[:, 0:304])

            # ---------------- stage 2: edges ----------------
            aggT = cp.tile([CB, 9, M], f32)
            for t in range(4):
                rs = slice(t * 128, (t + 1) * 128)
                nbt = sb.tile([128, K], i32, tag="nbt")
                nc.sync.dma_start(out=nbt[:], in_=nb_in[rs, :])
                sft = sb.tile([128, 1], i32, tag="sft")
                nc.sync.dma_start(out=sft[:], in_=self_idx[rs, :])
                ebt = sb1.tile([128, K * H], f32, tag="ebt")
                nc.sync.dma_start(out=ebt[:], in_=ebias[rs, :])

                gself = sb.tile([128, REC], f32, tag="gself")
                nc.gpsimd.indirect_dma_start(
                    out=gself[:], out_offset=None, in_=table_d[:],
                    in_offset=bass.IndirectOffsetOnAxis(ap=sft[:, :1], axis=0))

                ghs = []
                for hf in range(_NG):
                    gh = gp.tile([128, _KHALF, REC], f32, tag="g")
                    for kk in range(_KHALF):
                        k = hf * _KHALF + kk
                        nc.gpsimd.indirect_dma_start(
                            out=gh[:, kk, :], out_offset=None, in_=table_d[:],
                            in_offset=bass.IndirectOffsetOnAxis(ap=nbt[:, k:k + 1], axis=0))
                    ghs.append(gh)

                # logits [128, K, H] = q_gathered + s_self + ebias
                Lt = sb1.tile([128, K, H], f32, tag="Lt")
                sview = gself[:, 296:304].unsqueeze(1).broadcast_to([128, _KHALF, H])
                for hf in range(_NG):
                    nc.vector.tensor_add(out=Lt[:, hf * _KHALF:(hf + 1) * _KHALF, :],
                                         in0=ghs[hf][:, :, 0:H], in1=sview)
                nc.vector.tensor_add(out=Lt[:], in0=Lt[:],
                                     in1=ebt[:].rearrange("p (k h) -> p k h", k=K))

                mx = sb1.tile([128, H], f32, tag="mx")
                nc.vector.tensor_reduce(out=mx[:],
                                        in_=Lt[:].rearrange("p k h -> p h k"),
                                        axis=AX.X, op=OP.max)
                ex = sb1.tile([128, K, H], f32, tag="ex")
                nc.vector.tensor_tensor(out=ex[:], in0=Lt[:],
                                        in1=mx[:].unsqueeze(1).broadcast_to([128, K, H]),
                                        op=OP.subtract)
                nc.scalar.activation(out=ex[:], in_=ex[:], func=AF.Exp)
                sm = sb1.tile([128, H], f32, tag="sm")
                nc.vector.tensor_reduce(out=sm[:],
                                        in_=ex[:].rearrange("p k h -> p h k"),
                                        axis=AX.X, op=OP.add)
                nc.vector.tensor_scalar(sm[:], sm[:], 1e-9, scalar2=None, op0=OP.add)
                rc = sb1.tile([128, H], f32, tag="rc")
                nc.vector.reciprocal(out=rc[:], in_=sm[:])
                al = sb1.tile([128, K, H], f32, tag="al")
                nc.vector.tensor_tensor(out=al[:], in0=ex[:],
                                        in1=rc[:].unsqueeze(1).broadcast_to([128, K, H]),
                                        op=OP.mult)
                al32 = sb1.tile([128, K, CB], f32, tag="al32")
                nc.vector.tensor_copy(
                    out=al32[:].rearrange("p k (h c) -> p k h c", h=H),
                    in_=al[:].unsqueeze(3).broadcast_to([128, K, H, 4]))

                # weighted aggregation over k
                agg = sb1.tile([128, 288], f32, tag="agg")
                ahalf = sb1.tile([128, 288], f32, tag="ahalf")
                for hf in range(_NG):
                    alv = al32[:, hf * _KHALF:(hf + 1) * _KHALF, :]
                    gv = ghs[hf][:, :, H:296].rearrange("p k (m w) -> p k m w", m=9)
                    nc.vector.tensor_tensor(
                        out=gv, in0=gv,
                        in1=alv.unsqueeze(2).broadcast_to([128, _KHALF, 9, CB]),
                        op=OP.mult)
                    dst = agg if hf == 0 else ahalf
                    nc.vector.tensor_reduce(
                        out=dst[:],
                        in_=ghs[hf][:, :, H:296].rearrange("p k j -> p j k"),
                        axis=AX.X, op=OP.add)
                    if hf > 0:
                        nc.vector.tensor_add(out=agg[:], in0=agg[:], in1=ahalf[:])

                # transpose agg -> aggT[:, m, own-slice]
                for m in range(9):
                    ptr = psC.tile([CB, 128], f32, tag="small")
                    nc.tensor.transpose(out=ptr[:], in_=agg[:, m * CB:(m + 1) * CB],
                                        identity=ident[:])
                    nc.vector.tensor_copy(out=aggT[:, m, rs], in_=ptr[:])

            # ---------------- stage 3: outputs (feature-major) ----------------
            outT = cp.tile([CB, 9, M], f32, tag="bigA")
            for m in range(9):
                po = psB.tile([CB, M], f32, tag="b512")
                nc.tensor.matmul(po[:], lhsT=w_Wo[LMAP[m]][:], rhs=aggT[:, m, :],
                                 start=True, stop=True)
                if m == 0:
                    nc.vector.tensor_add(out=outT[:, 0, :], in0=po[:],
                                         in1=w_bo0[:].to_broadcast([CB, M]))
                else:
                    nc.vector.tensor_copy(out=outT[:, m, :], in_=po[:])

            # FFN on m=0
            ph = psB.tile([CB, M], f32, tag="b512")
            nc.tensor.matmul(ph[:], lhsT=w_Wf1[:], rhs=outT[:, 0, :], start=True, stop=True)
            h1 = sb1.tile([CB, M], f32, tag="h1")
            nc.scalar.activation(out=h1[:], in_=ph[:], func=AF.Relu, bias=w_bf1[:, :1])
            pf = psB.tile([CB, M], f32, tag="b512")
            nc.tensor.matmul(pf[:], lhsT=w_Wf2[:], rhs=h1[:], start=True, stop=True)
            f2 = sb1.tile([CB, M], f32, tag="f2")
            nc.vector.tensor_add(out=f2[:], in0=pf[:], in1=w_bf2[:].to_broadcast([CB, M]))
            nc.vector.tensor_add(out=outT[:, 0, :], in0=outT[:, 0, :], in1=f2[:])

            # gate & coordinate update
            upd_a = []
            for a in range(3):
                pu = psC.tile([1, M], f32, tag="small")
                nc.tensor.matmul(pu[:], lhsT=w_Wx1[:], rhs=outT[:, 1 + a, :],
                                 start=True, stop=True)
                ut = sb1.tile([1, M], f32, tag=f"upd{a}")
                nc.vector.tensor_copy(out=ut[:], in_=pu[:])
                upd_a.append(ut)
            pg = psC.tile([1, M], f32, tag="small")
            nc.tensor.matmul(pg[:], lhsT=w_Wg[:], rhs=outT[:, 0, :], start=True, stop=True)
            gT = sb1.tile([1, M], f32, tag="gT")
            nc.scalar.activation(out=gT[:], in_=pg[:], func=AF.Exp, bias=w_bg[:1, :1])
            nc.vector.tensor_scalar(gT[:], gT[:], 1.0, scalar2=None, op0=OP.add)
            nc.scalar.activation(out=gT[:], in_=gT[:], func=AF.Ln)

            for a in range(3):
                xu = sb1.tile([1, M], f32, tag=f"xu{a}")
                nc.vector.tensor_tensor(out=xu[:], in0=upd_a[a][:], in1=gT[:1, :],
                                        op=OP.mult)
                nc.vector.tensor_tensor(out=xu[:], in0=xu[:], in1=nm_own[:1, :],
                                        op=OP.mult)
                nc.vector.tensor_add(out=xu[:], in0=xu[:], in1=xo_a[a][:])
                nc.sync.dma_start(out=XT_out[a:a + 1, :], in_=xu[:])

            # backbone update
            for a in range(3):
                pb = psC.tile([3, M], f32, tag="small")
                nc.tensor.matmul(pb[:], lhsT=w_Wb1[:], rhs=outT[:, 1 + a, :],
                                 start=True, stop=True)
                ub = sb1.tile([3, M], f32, tag="ub")
                nc.vector.tensor_tensor(out=ub[:], in0=pb[:], in1=nm_own[:],
                                        op=OP.mult)
                nc.vector.tensor_add(out=ub[:], in0=ub[:], in1=bo_a[a][:])
                nc.sync.dma_start(out=bbT_out[a], in_=ub[:])

            nc.sync.dma_start(out=featsT_out[:].rearrange("m d n -> d m n"), in_=outT[:])

    nc.compile()
    return nc


def _get_nc():
    if "nc" not in _CACHE:
        _CACHE["nc"] = _build_kernel()
    return _CACHE["nc"]


# ----------------------------------------------------------------------------
# host-side exact reference pieces (jax CPU)
# ----------------------------------------------------------------------------

def _host_mod():
    if "host" in _CACHE:
        return _CACHE["host"]
    import jax
    import jax.numpy as jnp
    cpu = jax.devices("cpu")[0]
    _CACHE["host"] = (jax, jnp, cpu)
    return _CACHE["host"]


def _sample_edges_host(X, x_mask, layer_i):
    """Exact replica of reference.sample_edges, local indices [B, L, K]."""
    jax, jnp, cpu = _host_mod()
    with jax.default_device(cpu):
        key = jax.random.fold_in(jax.random.key(42), layer_i)
        Xb = jnp.where(x_mask[:, None], 1e9, X).reshape(B, L, 3)

        def per(Xp, k):
            d = jnp.linalg.norm(Xp[:, None] - Xp[None], axis=-1)
            idx = jnp.argsort(d, axis=-1)
            sd = jnp.take_along_axis(d, idx, -1)
            knn = idx[:, :KNN]
            u = jax.random.uniform(k, (L, L - KNN), minval=1e-6, maxval=1.0 - 1e-6)
            logp = -3.0 * jnp.log(jnp.maximum(sd[:, KNN:], 1e-9)) - jnp.log(-jnp.log(u))
            _, top = jax.lax.top_k(logp, INV)
            samp = jnp.take_along_axis(idx[:, KNN:], top, -1)
            return jnp.concatenate([knn, samp], -1)

        nb = jax.vmap(per)(Xb, jax.random.split(key, B))
        return np.asarray(nb).astype(np.int32)       # [B, L, K] local


def _edge_bias_host(X, nb_local, We_i, be_i, Wa3_i, ba_i):
    """ebias[n,k,h] = relu([rbf|posemb] @ We + be) @ Wa[70:] + ba, with -1e9
    folded in for invalid edges. X: [N,3] centered; nb_local: [B,L,K]."""
    jax, jnp, cpu = _host_mod()
    with jax.default_device(cpu):
        nbg = (nb_local.astype(np.int64)
               + (np.arange(B)[:, None, None] * L)).reshape(-1)
        slf = np.repeat(np.arange(N), K)
        Xj = jnp.asarray(X)
        dvec = Xj[nbg] - Xj[slf]
        dist = jnp.linalg.norm(dvec, axis=-1)
        valid = (dist > 0.1) & (dist < 1e8)
        mu = jnp.linspace(0.0, 20.0, 16)
        sig = 20.0 / 16.0
        rbf = jnp.exp(-(((dist[:, None] - mu) / sig) ** 2))
        freq = jnp.exp(jnp.arange(0, 16, 2, dtype=jnp.float32)
                       * (-np.log(10000.0) / 16.0))
        diff = (nbg - slf).astype(np.int32)
        aa = jnp.asarray(diff)[:, None].astype(jnp.float32) * freq
        pe = jnp.concatenate([jnp.cos(aa), jnp.sin(aa)], -1)
        e = jax.nn.relu(jnp.concatenate([rbf, pe], -1) @ jnp.asarray(We_i)
                        + jnp.asarray(be_i))
        eb = e @ jnp.asarray(Wa3_i) + jnp.asarray(ba_i)
        eb = jnp.where(valid[:, None], eb, -1e9)
        return np.asarray(eb, dtype=np.float32).reshape(B, L, K * H)


def kernel(noised_bb, t, x_mask, noising_mask, kappa, tW1, tb1, tW2, tb2, eW, eb,
           We, be, Wa, ba, Wv, bv, Wo, bo, Wf1, bf1, Wf2, bf2, Wx, bx, Wg, bg,
           Wb, bbias):
    import os
    os.environ["BASS_NEVER_TRACE"] = "1"   # no NTFF hook on this axon client
    from concourse.bass_utils import run_bass_kernel_spmd

    jax, jnp, cpu = _host_mod()
    nc = _get_nc()

    noised_bb = np.asarray(noised_bb, dtype=np.float32)
    x_mask_np = np.asarray(x_mask)
    nmask_np = np.asarray(noising_mask)

    with jax.default_device(cpu):
        X0 = jnp.asarray(noised_bb[:, 1])
        w = (~jnp.asarray(x_mask_np)).astype(jnp.float32).reshape(B, L, 1)
        Xr = X0.reshape(B, L, 3)
        center = jnp.repeat((Xr * w).sum(1) / jnp.maximum(w.sum(1), 1.0), L, axis=0)
        X = np.asarray(X0 - center, dtype=np.float32)          # [N,3]
        tp = 2.0 * np.pi * jnp.asarray(t)[:, None] * jnp.asarray(kappa)
        ft = jnp.concatenate([jnp.cos(tp), jnp.sin(tp)], -1)
        et = jax.nn.relu(jax.nn.relu(ft @ jnp.asarray(tW1) + jnp.asarray(tb1))
                         @ jnp.asarray(tW2) + jnp.asarray(tb2))   # [B,64]
        tvec_np = np.asarray(et @ jnp.asarray(eW)[CB:] + jnp.asarray(eb),
                             dtype=np.float32)                  # [B,32]
    center_np = np.asarray(center, dtype=np.float32)

    bb_rel = noised_bb[:, [0, 2, 3]]                            # [N,3,3]
    # device states (per protein)
    featsT = [np.zeros((9, CB, L), np.float32) for _ in range(B)]
    bbT = [np.ascontiguousarray(bb_rel.reshape(B, L, 3, 3)[p].transpose(2, 1, 0))
           for p in range(B)]                                   # [a, j, n]
    XT = [np.ascontiguousarray(X.reshape(B, L, 3)[p].T) for p in range(B)]
    nmask_f = nmask_np.astype(np.float32).reshape(B, L)

    Wa_np = np.asarray(Wa, dtype=np.float32)
    core_ids = list(range(8))

    for i in range(NL):
        nb_local = _sample_edges_host(X, jnp.asarray(x_mask_np), i)  # [B,L,K]
        ebias_np = _edge_bias_host(X, nb_local,
                                   np.asarray(We)[i], np.asarray(be)[i],
                                   Wa_np[i][2 * SPH:], np.asarray(ba)[i])
        in_maps = []
        for c in core_ids:
            p, half = c // 2, c % 2
            sl = slice(half * M, (half + 1) * M)
            im = {
                "featsT": featsT[p],
                "nmask_full": nmask_f[p][None, :],
                "nmask_own": np.repeat(nmask_f[p][None, sl], 3, axis=0),
                "bbT_full": bbT[p],
                "bb_own": np.ascontiguousarray(bbT[p][:, :, sl]),
                "XT_own": np.ascontiguousarray(XT[p][:, sl]),
                "tvec": tvec_np[p][:, None],
                "nb": np.ascontiguousarray(nb_local[p, sl]),
                "self_idx": np.arange(half * M, (half + 1) * M,
                                      dtype=np.int32)[:, None],
                "ebias": np.ascontiguousarray(ebias_np[p, sl]),
                "eWf": np.asarray(eW, np.float32)[:CB],
                "Wq": Wa_np[i][:SPH],
                "Ws": Wa_np[i][SPH:2 * SPH],
                "Wf1": np.asarray(Wf1, np.float32)[i],
                "Wf2": np.asarray(Wf2, np.float32)[i],
                "Wx1": np.asarray(Wx, np.float32)[i][1],
                "Wg": np.asarray(Wg, np.float32)[i],
                "Wb1": np.asarray(Wb, np.float32)[i][1],
                "bv_b": np.repeat(np.asarray(bv, np.float32)[i][None, :], 128, 0),
                "bo0": np.asarray(bo, np.float32)[i][:, None],
                "bf1": np.asarray(bf1, np.float32)[i][:, None],
                "bf2": np.asarray(bf2, np.float32)[i][:, None],
                "bg1": np.asarray(bg, np.float32)[i].reshape(1, 1),
            }
            for l in range(3):
                im[f"Wv{l}"] = np.asarray(Wv, np.float32)[i][l]
                im[f"Wo{l}"] = np.asarray(Wo, np.float32)[i][l]
            in_maps.append(im)

        res = run_bass_kernel_spmd(nc, in_maps, core_ids=core_ids)
        _CACHE.setdefault("results", []).append(res)
        for c in core_ids:
            p, half = c // 2, c % 2
            sl = slice(half * M, (half + 1) * M)
            r = res.results[c]
            featsT[p][:, :, sl] = r["featsT_out"]
            XT[p][:, sl] = r["XT_out"]
            bbT[p][:, :, sl] = r["bbT_out"]
        X = np.concatenate([XT[p].T for p in range(B)], axis=0)

    den = np.zeros((N, 4, 3), np.float32)
    den[:, 1] = X + center_np
    bb_final = np.concatenate(
        [bbT[p].transpose(2, 1, 0) for p in range(B)], axis=0)  # [N, j, a]
    den[:, 0] = bb_final[:, 0]
    den[:, 2] = bb_final[:, 1]
    den[:, 3] = bb_final[:, 2]
    return den
